# revision 24
# baseline (speedup 1.0000x reference)
"""Trainium2 Bass kernel for nn_C2D_34419867910289.

Computation (per feature j of 32, batch B=4096):
  q = cat_j @ Wq_j ; k = emb_j @ Wk_j ; v = emb_j @ Wv_j
  alpha = softmax(q k^T / sqrt(D)) ; h = LN1(cat_j + alpha v)
  h2 = LN2(h + relu(h W1 + b1) W2 + b2) ; out = sigmoid(h2 . Ws_j + bs_j)

Sharding: Nc (feature) axis across 8 cores, 4 features/core, full batch.
Feature-major dataflow: activations live as [D=128 partitions, Bt=512 free].

v2 design (vs v1 baseline):
 - Single activation-table set for the whole main loop: LN rstd computed as
   exp(-0.5*ln(var+eps)), so Exp/Ln/Relu/Copy all come from
   natural_log_exp_and_others (one ACT_TABLE_LOAD, plus one for the final
   Sigmoid).
 - Scores for both c-chunks land in one 2-bank PSUM slot -> single Exp
   ACTIVATE (FD=1024) per feature; same for ff1+Relu.
 - PSUM slot reuse: hu accumulates into the scores slot half after Exp
   drains it; ff2 accumulates into the ff1 slot half after Relu.
 - All per-tile stats share ONE PSUM bank in 32-row strips with temporal
   reuse: mu1@0-3, sq1@32-35, se(pair0)@64-67, then mu2@0-3/wsy@4-7,
   sq2@32-35.  se(pair1) borrows a broadcast-pool slot's rows 64-67.
 - scalar_tensor_tensor fusions: x1ln = (zc*g1)*rsb, x2 = (x1ln+bb)+ff2.
 - LN2 stats staged to SBUF via DMA directly from PSUM (no ScalarE copies).
 - Deferred LN2 + sigmoid batched chain at kernel end (as v1).
"""

import os
import sys

import numpy as np

sys.path.insert(0, "/opt/trn_rl_repo")

import ml_dtypes

BF16 = ml_dtypes.bfloat16

B, NC, D, C, H = 4096, 32, 128, 256, 256
NCORES = 8
FPC = NC // NCORES  # features per core = 4
BT = 512            # batch tile (matmul moving free dim)
NT = B // BT        # 8 b-tiles
EPS = 1e-5
ISCALE = 1.0 / np.sqrt(np.float32(D))

_CACHE = {}
LAST = {}  # exec_time_ns etc. for test harness


def _build_program(zb):
    """Emit the SPMD per-core Bass/Tile program (identical on all cores)."""
    import concourse.bacc as bacc
    import concourse.bass as bass
    import concourse.tile as tile
    from concourse import bass_isa, mybir

    f32 = mybir.dt.float32
    bf16 = mybir.dt.bfloat16
    AF = mybir.ActivationFunctionType
    OP = mybir.AluOpType

    nc = bacc.Bacc("TRN2", target_bir_lowering=False, debug=False)

    # ---- DRAM I/O (per-core shards) ----
    catT_d = nc.dram_tensor("catT", [FPC * D, B], bf16, kind="ExternalInput")
    embT_d = nc.dram_tensor("embT", [FPC * D, C], bf16, kind="ExternalInput")
    wqT_d = nc.dram_tensor("wqT", [FPC * D, D], bf16, kind="ExternalInput")
    wk_d = nc.dram_tensor("wk", [FPC * D, D], bf16, kind="ExternalInput")
    wv_d = nc.dram_tensor("wv", [FPC * D, D], bf16, kind="ExternalInput")
    w1_d = nc.dram_tensor("w1", [FPC * D, H], bf16, kind="ExternalInput")
    w2_d = nc.dram_tensor("w2", [FPC * H, D], bf16, kind="ExternalInput")
    wsT_d = nc.dram_tensor("wsT", [D, FPC], bf16, kind="ExternalInput")
    g1b1_d = nc.dram_tensor("g1b1", [D, 2], f32, kind="ExternalInput")
    g2_d = nc.dram_tensor("g2", [D, 1], f32, kind="ExternalInput")
    beta2_d = nc.dram_tensor("beta2", [D, 1], bf16, kind="ExternalInput")
    b1_d = nc.dram_tensor("b1", [FPC, H], f32, kind="ExternalInput")
    b2_d = nc.dram_tensor("b2", [FPC, D], f32, kind="ExternalInput")
    bs_d = nc.dram_tensor("bs", [FPC, 1], f32, kind="ExternalInput")
    bcm_d = nc.dram_tensor("bcm", [4, FPC * D], bf16, kind="ExternalInput")
    bcm2_d = nc.dram_tensor("bcm2", [2, 2 * D], bf16, kind="ExternalInput")
    bcmg_d = nc.dram_tensor("bcmg", [4, FPC * D], bf16, kind="ExternalInput")
    out_d = nc.dram_tensor("out", [FPC, B], f32, kind="ExternalOutput")

    with tile.TileContext(nc) as tc:
        with (
            tc.tile_pool(name="const", bufs=1) as constp,
            tc.tile_pool(name="wtmp", bufs=2) as wtmp,
            tc.tile_pool(name="cat", bufs=12) as catp,
            tc.tile_pool(name="et", bufs=4) as etp,
            tc.tile_pool(name="x1p", bufs=6) as x1p,
            tc.tile_pool(name="xln", bufs=3) as xlnp,
            tc.tile_pool(name="rp", bufs=2) as rp,
            tc.tile_pool(name="x2p", bufs=3) as x2p,
            tc.tile_pool(name="tmp", bufs=4) as tmpp,
            tc.tile_pool(name="stash", bufs=4) as stashp,
            tc.tile_pool(name="finp", bufs=1) as finp,
            tc.tile_pool(name="sp", bufs=2, space="PSUM") as spp,
            tc.tile_pool(name="bc", bufs=2, space="PSUM") as bcp,
            tc.tile_pool(name="pst", bufs=2, space="PSUM") as pstp,
        ):
            # ---------------- constants ----------------
            ones_c1 = constp.tile([D, 1], bf16, tag="c_ones")
            nc.vector.memset(ones_c1, 1.0)
            epsT = constp.tile([D, 1], f32, tag="c_eps")
            nc.vector.memset(epsT, EPS)

            # masked stat lhsTs (per feature j):
            # m4[j]:  [D,4] col j = 1/128   -> mu rows
            # se4[j]: [D,4] col j = 1.0     -> sumexp rows
            # mw8[j]: [D,8] col j = 1/128, col 4+j = Ws*g2 -> mu2 rows 0..3,
            #         wsy rows 4..7 (one matmul on x2)
            m4, se4, mw8 = [], [], []
            for j in range(FPC):
                t = constp.tile([D, 4], bf16, tag=f"c_m4_{j}")
                nc.vector.memset(t, 0.0)
                nc.vector.memset(t[:, j : j + 1], 1.0 / 128.0)
                m4.append(t)
                t = constp.tile([D, 4], bf16, tag=f"c_se4_{j}")
                nc.vector.memset(t, 0.0)
                nc.vector.memset(t[:, j : j + 1], 1.0)
                se4.append(t)
                t = constp.tile([D, 8], bf16, tag=f"c_mw8_{j}")
                nc.vector.memset(t, 0.0)
                nc.vector.memset(t[:, j : j + 1], 1.0 / 128.0)
                mw8.append(t)
            # M=36 zero-init variant of the mu mask: first matmul of each LN1
            # bank clears rows 0..35 so accumulates are defined
            m36_0 = constp.tile([D, 36], bf16, tag="c_m36")
            nc.vector.memset(m36_0, 0.0)
            nc.vector.memset(m36_0[:, 0:1], 1.0 / 128.0)

            # one-hot row-broadcast lhsT (row j ones in [4, D] slice j)
            bcm = constp.tile([4, FPC * D], bf16, tag="c_bcm")
            nc.sync.dma_start(bcm, bcm_d[:, :])

            def bc(j):
                return bcm[:, j * D : (j + 1) * D]

            bcm2 = constp.tile([2, 2 * D], bf16, tag="c_bcm2")
            nc.sync.dma_start(bcm2, bcm2_d[:, :])
            bcmg = constp.tile([4, FPC * D], bf16, tag="c_bcmg")
            nc.sync.dma_start(bcmg, bcmg_d[:, :])

            def bcg(j):
                return bcmg[:, j * D : (j + 1) * D]

            def bc2(r):
                return bcm2[:, r * D : (r + 1) * D]

            def rowbc(row_ap):
                """[1,BT] SBUF row -> [1,D,BT] AP replicating the row D times
                via a 0-stride middle dim (DMA row-broadcast source)."""
                return bass.AP(
                    tensor=row_ap.tensor, offset=row_ap.offset,
                    ap=[list(row_ap.ap[0]), [0, D], [1, BT]],
                )

            # small param cols
            g1b1 = constp.tile([D, 2], f32, tag="c_g1b1")
            nc.sync.dma_start(g1b1, g1b1_d[:, :])
            negg1d = constp.tile([D, 1], f32, tag="c_negg1d")
            nc.vector.tensor_scalar(
                negg1d, g1b1[:, 0:1], -1.0 / 128.0, None, OP.mult
            )
            g2c = constp.tile([D, 1], f32, tag="c_g2")
            nc.sync.dma_start(g2c, g2_d[:, :])
            beta2c = constp.tile([D, 1], bf16, tag="c_beta2")
            nc.sync.dma_start(beta2c, beta2_d[:, :])
            wsT = constp.tile([D, FPC], bf16, tag="c_wsT")
            nc.sync.dma_start(wsT, wsT_d[:, :])
            bs_sb = constp.tile([FPC, 1], f32, tag="c_bs")
            nc.sync.dma_start(bs_sb, bs_d[:, :])
            Scol = constp.tile([4, 1], f32, tag="c_Scol")
            Tcol = constp.tile([4, 1], f32, tag="c_Tcol")
            NH = 2 * NT  # rows per half (4 features x NT/2 tiles)
            Scol32 = [
                constp.tile([NH, 1], f32, tag=f"c_Scol32_{h}", name=f"Scol32_{h}") for h in range(2)
            ]
            Tcol32 = [
                constp.tile([NH, 1], f32, tag=f"c_Tcol32_{h}", name=f"Tcol32_{h}") for h in range(2)
            ]

            # packed deferred-LN2 stats, split in half-kernel chunks;
            # row index = 4*(t % (NT/2)) + j within each half
            NR = 4 * NT
            fin_mu2 = [finp.tile([NH, BT], f32, tag=f"fin_mu2_{h}", name=f"fin_mu2_{h}") for h in range(2)]
            fin_wsy = [finp.tile([NH, BT], f32, tag=f"fin_wsy_{h}", name=f"fin_wsy_{h}") for h in range(2)]
            fin_q = [finp.tile([NH, BT], f32, tag=f"fin_q_{h}", name=f"fin_q_{h}") for h in range(2)]

            # ---------------- per-feature setup ----------------
            mq_s, v_s, w1_s, w2_s, c1_s, bb_s = [], [], [], [], [], []
            for j in range(FPC):
                r0 = j * D
                w1 = constp.tile([D, H], bf16, tag=f"w1{j}")
                nc.scalar.dma_start(w1, w1_d[r0 : r0 + D, :])
                w1_s.append(w1)
                w2 = constp.tile([D, 2, D], bf16, tag=f"w2{j}")
                nc.scalar.dma_start(w2[:, 0, :], w2_d[j * H : j * H + D, :])
                nc.scalar.dma_start(w2[:, 1, :], w2_d[j * H + D : j * H + 2 * D, :])
                w2_s.append(w2)
                b1c = wtmp.tile([D, 2], f32, tag="b1t")
                nc.sync.dma_start(
                    b1c, bass.AP(tensor=b1_d, offset=j * H, ap=[[1, D], [D, 2]])
                )
                b2c = wtmp.tile([D, 1], f32, tag="b2t")
                nc.sync.dma_start(
                    b2c, bass.AP(tensor=b2_d, offset=j * D, ap=[[1, D], [D, 1]])
                )
                bb = constp.tile([D, 1], f32, tag=f"bb{j}")
                nc.vector.tensor_add(bb, g1b1[:, 1:2], b2c)  # beta1 + b2
                bb_s.append(bb)

                embT = wtmp.tile([D, C], bf16, tag="embT")
                nc.gpsimd.dma_start(embT, embT_d[r0 : r0 + D, :])
                wk = wtmp.tile([D, D], bf16, tag="wk")
                nc.gpsimd.dma_start(wk, wk_d[r0 : r0 + D, :])
                wv = wtmp.tile([D, D], bf16, tag="wv")
                nc.scalar.dma_start(wv, wv_d[r0 : r0 + D, :])
                wqT = wtmp.tile([D, D], bf16, tag="wqT")
                nc.gpsimd.dma_start(wqT, wqT_d[r0 : r0 + D, :])

                # beta1 column in bf16 for the c1 matvec
                be1_16 = wtmp.tile([D, 1], bf16, tag="be1_16")
                nc.vector.tensor_copy(be1_16, g1b1[:, 1:2])

                # kT = Wk.T @ embT -> [E, C], scaled by 1/sqrt(D)
                kps = spp.tile([D, 2, BT], f32, tag="sp")
                nc.tensor.matmul(kps[:, 0, :C], wk, embT, start=True, stop=True)
                kts = wtmp.tile([D, C], bf16, tag="kts")
                nc.scalar.activation(kts, kps[:, 0, :C], AF.Copy, scale=float(ISCALE))

                # M_j = Wq_j @ kts -> [D, C]; scores^T = M_j.T @ cat^T
                mps = spp.tile([D, 2, BT], f32, tag="sp")
                nc.tensor.matmul(mps[:, 0, :C], wqT, kts, start=True, stop=True)
                mq = constp.tile([D, C], bf16, tag=f"mq{j}")
                nc.scalar.activation(mq, mps[:, 0, :C], AF.Copy)
                mq_s.append(mq)

                # v chunks: [c-chunk=128, E]
                vt = constp.tile([D, 2, D], bf16, tag=f"v{j}")
                vps = spp.tile([D, 2, BT], f32, tag="sp")
                for c in range(2):
                    nc.tensor.matmul(
                        vps[:, c, :D], embT[:, c * D : (c + 1) * D], wv,
                        start=True, stop=True,
                    )
                nc.scalar.activation(vt, vps[:, :, :D], AF.Copy)
                v_s.append(vt)

                # c1[hc] = b1[hc] + W1_hc^T beta1  (relu bias, per h-chunk)
                c1ps = bcp.tile([D, BT], f32, tag="bc")
                for hc in range(2):
                    nc.tensor.matmul(
                        c1ps[:, hc : hc + 1],
                        w1[:, hc * D : (hc + 1) * D], be1_16,
                        start=True, stop=True,
                    )
                c1c = constp.tile([D, 2], f32, tag=f"c1{j}")
                nc.vector.tensor_add(c1c, b1c, c1ps[:, 0:2])
                c1_s.append(c1c)

            # Wsg2 = Ws*g2 ; S_j = sum_d Wsg2_j ; T_j = Ws_j.beta2 + bs_j
            wsg2_4 = constp.tile([D, FPC], bf16, tag="c_wsg2")
            nc.vector.tensor_scalar(wsg2_4, wsT, g2c, None, OP.mult)
            for j in range(FPC):
                nc.gpsimd.tensor_copy(mw8[j][:, 4 + j : 5 + j], wsg2_4[:, j : j + 1])
            sps = bcp.tile([FPC, BT], f32, tag="bc")
            nc.tensor.matmul(sps[:, :1], wsg2_4, ones_c1, start=True, stop=True)
            nc.scalar.activation(Scol, sps[:, :1], AF.Copy)
            tps = bcp.tile([FPC, BT], f32, tag="bc")
            nc.tensor.matmul(tps[:, :1], wsT, beta2c, start=True, stop=True)
            tcol0 = constp.tile([FPC, 1], f32, tag="c_T0")
            nc.scalar.activation(tcol0, tps[:, :1], AF.Copy)
            nc.gpsimd.tensor_add(Tcol, tcol0, bs_sb)
            for t in range(NT):
                h, r = t // (NT // 2), 4 * (t % (NT // 2))
                nc.sync.dma_start(Scol32[h][r : r + 4, :], Scol)
                nc.sync.dma_start(Tcol32[h][r : r + 4, :], Tcol)

            # deferred LN2 + sigmoid for half h of the b-tiles
            def fin_chain(h):
                n = NH
                musq2 = stashp.tile([n, BT], f32, tag="musq2")
                nc.vector.tensor_mul(musq2, fin_mu2[h], fin_mu2[h])
                var2 = stashp.tile([n, BT], f32, tag="var2")
                nc.vector.tensor_sub(var2, fin_q[h], musq2)
                std2 = stashp.tile([n, BT], f32, tag="std2")
                nc.scalar.activation(std2, var2, AF.Sqrt, bias=epsT[0:n, :])
                rstd2 = stashp.tile([n, BT], f32, tag="rstd2")
                nc.vector.reciprocal_approx_fast(rstd2, std2)
                mu2S = stashp.tile([n, BT], f32, tag="mu2S")
                nc.vector.tensor_scalar(
                    mu2S, fin_mu2[h], Scol32[h], None, OP.mult
                )
                t1 = stashp.tile([n, BT], f32, tag="t1")
                nc.vector.tensor_sub(t1, fin_wsy[h], mu2S)
                t2 = stashp.tile([n, BT], f32, tag="t2")
                nc.vector.tensor_mul(t2, t1, rstd2)
                o32 = stashp.tile([n, BT], f32, tag="o32")
                nc.scalar.activation(o32, t2, AF.Sigmoid, bias=Tcol32[h])
                # row 4t'+j -> out[j, 512t : 512t+512]
                out_ap = bass.AP(
                    tensor=out_d, offset=h * (NT // 2) * BT,
                    ap=[[BT, NT // 2], [B, FPC], [1, BT]],
                )
                nc.sync.dma_start(out_ap, o32)

            # ---------------- main loop over b-tiles ----------------
            for t in range(NT):
                b0 = t * BT
                # stats bank strips (temporal reuse):
                #   rows 0-3 mu1 (m36 clears 0-35), rows 32-35 sq1,
                #   rows 64-67 se(pair0); later mu2@0-3/wsy@4-7, sq2@32-35
                stats = pstp.tile([D, BT], f32, tag="st")

                cat_sb = [None] * FPC
                mu_t = [None] * FPC
                et_sb = [None] * FPC
                sp_sl = [None] * FPC
                x1_sb = [None] * FPC
                se_ps = [None] * 2      # psum holding se rows for pair p
                se_r0 = [0, 0]          # row offset of se rows in that psum
                srows = [None] * 2      # [4,BT] SBUF with s at rows 2p..2p+1

                # ---- phase A: scores, exp, se, hu (per feature) ----
                for j in range(FPC):
                    p = j // 2
                    ct = catp.tile([D, BT], bf16, tag="cat")
                    nc.gpsimd.dma_start(
                        ct, catT_d[j * D : (j + 1) * D, b0 : b0 + BT]
                    )
                    cat_sb[j] = ct
                    slot = spp.tile([D, 2, BT], f32, tag="sp")
                    sp_sl[j] = slot
                    if j == 0:
                        se_ps[0] = stats
                        se_r0[0] = 64
                    elif j == 2:
                        # pair1 sumexp -> stats rows 96-99 (strip 3, unused)
                        se_ps[1] = stats
                        se_r0[1] = 96
                    for c in range(2):
                        nc.tensor.matmul(
                            slot[:, c, :], mq_s[j][:, c * D : (c + 1) * D], ct,
                            start=True, stop=True,
                        )
                    et = etp.tile([D, 2, BT], bf16, tag="et")
                    nc.scalar.activation(et, slot, AF.Exp)
                    et_sb[j] = et
                    r0 = se_r0[p]
                    for c in range(2):
                        nc.tensor.matmul(
                            se_ps[p][r0 : r0 + 4, :], se4[j % 2], et[:, c, :],
                            start=(j % 2 == 0 and c == 0),
                            stop=(j % 2 == 1 and c == 1),
                            tile_position=(0, r0),
                            skip_group_check=True,
                        )
                    # hu reuses the scores slot half c=0 (start clears it)
                    for c in range(2):
                        nc.tensor.matmul(
                            slot[:, 0, :], v_s[j][:, c, :], et[:, c, :],
                            start=(c == 0), stop=(c == 1),
                        )
                    if j % 2 == 1:
                        # s for this pair -> srows[p] [2,BT] (bf16)
                        sr = stashp.tile([2, BT], bf16, tag=f"sr{p}")
                        nc.vector.tensor_copy(sr, se_ps[p][r0 : r0 + 2, :])
                        srows[p] = sr

                # ---- phase B: x1 = cat*s + hu, LN1 stats (per feature) ----
                for j in range(FPC):
                    p = j // 2
                    sbb = bcp.tile([D, BT], f32, tag="bc")
                    nc.tensor.matmul(
                        sbb, bc2(j % 2), srows[p], start=True, stop=True
                    )
                    cs = tmpp.tile([D, BT], bf16, tag="cs")
                    nc.vector.tensor_mul(cs, cat_sb[j], sbb)
                    x1 = x1p.tile([D, BT], bf16, tag="x1")
                    nc.vector.tensor_add(x1, cs, sp_sl[j][:, 0, :])
                    x1_sb[j] = x1
                    if zb:
                        # LN1 variance eliminated (LN2 absorbs column scale)
                        nc.tensor.matmul(
                            stats[0:4, :], m4[j], x1,
                            start=(j == 0), stop=(j == FPC - 1),
                            skip_group_check=True,
                        )
                        continue
                    sq1 = tmpp.tile([D, BT], bf16, tag="sq1")
                    nc.gpsimd.tensor_mul(sq1, x1, x1)
                    if j == 0:
                        nc.tensor.matmul(
                            stats[0:36, :], m36_0, x1,
                            start=True, stop=False,
                            skip_group_check=True,
                        )
                    else:
                        nc.tensor.matmul(
                            stats[0:4, :], m4[j], x1,
                            start=False, stop=False,
                            skip_group_check=True,
                        )
                    nc.tensor.matmul(
                        stats[32:36, :], m4[j], sq1,
                        start=False, stop=(j == FPC - 1),
                        tile_position=(0, 32),
                        skip_group_check=True,
                    )

                # ---- LN1 batched stat chain ----
                muS = stashp.tile([4, BT], bf16, tag="muS")
                nc.scalar.activation(muS, stats[0:4, :], AF.Copy)
                if zb:
                    rstd = None
                else:
                    musq = stashp.tile([4, BT], bf16, tag="musq")
                    nc.vector.tensor_mul(musq, muS, muS)
                    var1 = stashp.tile([4, BT], f32, tag="var1")
                    nc.vector.tensor_sub(var1, stats[32:36, :], musq)
                    rstd = stashp.tile([4, BT], bf16, tag="rstd")
                    std1 = stashp.tile([4, BT], f32, tag="std1")
                    nc.scalar.activation(std1, var1, AF.Sqrt, bias=epsT[0:4, :])
                    rstdf = stashp.tile([4, BT], f32, tag="rstdf")
                    nc.vector.reciprocal_approx_fast(rstdf, std1)
                    nc.vector.tensor_copy(rstd, rstdf)

                # ---- phase C: LN1 apply, FFN, LN2 stats ----
                for j in range(FPC):
                    if zb:
                        # u = x1*g1 - mu*g1 = LN1(x1)*std1; ReLU is positively
                        # homogeneous and LN2 is column-scale invariant, so
                        # the std1 factor never needs to be applied.
                        mub = bcp.tile([D, BT], f32, tag="bc")
                        nc.tensor.matmul(mub, bcg(j), muS, start=True, stop=True)
                        u = xlnp.tile([D, BT], bf16, tag="x1ln")
                        nc.vector.scalar_tensor_tensor(
                            u, x1_sb[j], g1b1[:, 0:1], mub, OP.mult, OP.subtract
                        )
                        x1ln = u
                    else:
                        mub = bcp.tile([D, BT], f32, tag="bc")
                        nc.tensor.matmul(mub, bc(j), muS, start=True, stop=True)
                        rsb = bcp.tile([D, BT], f32, tag="bc")
                        nc.tensor.matmul(rsb, bc(j), rstd, start=True, stop=True)
                        zc = tmpp.tile([D, BT], bf16, tag="zc")
                        nc.vector.tensor_sub(zc, x1_sb[j], mub)
                        x1ln = xlnp.tile([D, BT], bf16, tag="x1ln")
                        nc.vector.scalar_tensor_tensor(
                            x1ln, zc, g1b1[:, 0:1], rsb, OP.mult, OP.mult
                        )
                    fslot = spp.tile([D, 2, BT], f32, tag="sp")
                    for hc in range(2):
                        nc.tensor.matmul(
                            fslot[:, hc, :], w1_s[j][:, hc * D : (hc + 1) * D],
                            x1ln, start=True, stop=True,
                        )
                    r_sb = rp.tile([D, 2, BT], bf16, tag="r")
                    if zb:
                        nc.scalar.activation(r_sb, fslot, AF.Relu)
                    else:
                        for hc in range(2):
                            nc.scalar.activation(
                                r_sb[:, hc, :], fslot[:, hc, :], AF.Relu,
                                bias=c1_s[j][:, hc : hc + 1],
                            )
                    f2ps = bcp.tile([D, BT], f32, tag="bc")
                    for hc in range(2):
                        nc.tensor.matmul(
                            f2ps, w2_s[j][:, hc, :], r_sb[:, hc, :],
                            start=(hc == 0), stop=(hc == 1),
                        )
                    x2 = x2p.tile([D, BT], bf16, tag="x2")
                    if zb:
                        # X = u + ff2 = x2 * std1 (LN2 absorbs the scale)
                        nc.vector.tensor_add(x2, x1ln, f2ps)
                    else:
                        nc.vector.scalar_tensor_tensor(
                            x2, x1ln, bb_s[j], f2ps, OP.add, OP.add
                        )
                    sq2 = tmpp.tile([D, BT], bf16, tag="sq2")
                    nc.gpsimd.tensor_mul(sq2, x2, x2)
                    # mu2 rows 0-3 / wsy rows 4-7 (strip 0 reuse after muS read)
                    nc.tensor.matmul(
                        stats[0:8, :], mw8[j], x2,
                        start=(j == 0), stop=(j == FPC - 1),
                        skip_group_check=True,
                    )
                    # sq2 rows 32-35 (strip 32 reuse after var1 read)
                    nc.tensor.matmul(
                        stats[32:36, :], m4[j], sq2,
                        start=(j == 0), stop=(j == FPC - 1),
                        tile_position=(0, 32),
                        skip_group_check=True,
                    )

                # stage LN2 stats to SBUF, then DMA into packed fin buffers
                stage = stashp.tile([8, BT], f32, tag="stage")
                nc.scalar.activation(stage, stats[0:8, :], AF.Copy)
                stage2 = stashp.tile([4, BT], f32, tag="stage2")
                nc.scalar.activation(stage2, stats[32:36, :], AF.Copy)
                h, r = t // (NT // 2), 4 * (t % (NT // 2))
                nc.sync.dma_start(fin_mu2[h][r : r + 4, :], stage[0:4, :])
                nc.sync.dma_start(fin_wsy[h][r : r + 4, :], stage[4:8, :])
                nc.sync.dma_start(fin_q[h][r : r + 4, :], stage2)
                if t == NT // 2 - 1:
                    fin_chain(0)
                elif t == NT - 1:
                    fin_chain(1)



    nc.compile()
    return nc


def _get_program(zb=True):
    key = f"nc{int(bool(zb))}"
    if key not in _CACHE:
        _CACHE[key] = _build_program(zb)
    return _CACHE[key]


def _shard_inputs(inputs):
    """Host-side layout prep: shard by feature, transpose, cast. No FLOPs."""
    cat = np.ascontiguousarray(np.asarray(inputs["cat_vecs"], dtype=np.float32))
    emb = np.asarray(inputs["embed_weights"], dtype=np.float32)
    wq = np.asarray(inputs["Wq"], dtype=np.float32)
    wk = np.asarray(inputs["Wk"], dtype=np.float32)
    wv = np.asarray(inputs["Wv"], dtype=np.float32)
    w1 = np.asarray(inputs["W1"], dtype=np.float32)
    w2 = np.asarray(inputs["W2"], dtype=np.float32)
    b1 = np.asarray(inputs["b1"], dtype=np.float32)
    b2 = np.asarray(inputs["b2"], dtype=np.float32)
    ws = np.asarray(inputs["Ws"], dtype=np.float32)
    bs = np.asarray(inputs["bs"], dtype=np.float32)
    g1 = np.asarray(inputs["ln1_g"], dtype=np.float32)
    be1 = np.asarray(inputs["ln1_b"], dtype=np.float32)
    g2 = np.asarray(inputs["ln2_g"], dtype=np.float32)
    be2 = np.asarray(inputs["ln2_b"], dtype=np.float32)

    g1b1 = np.ascontiguousarray(np.stack([g1, be1], axis=1))  # [D,2] f32
    g2c = np.ascontiguousarray(g2[:, None])
    be2c = be2[:, None].astype(BF16)
    bcm = np.zeros((4, FPC, D), dtype=np.float32)
    for j in range(FPC):
        bcm[j, j, :] = 1.0
    bcm = bcm.reshape(4, FPC * D).astype(BF16)
    bcm2 = np.zeros((2, 2, D), dtype=np.float32)
    for r in range(2):
        bcm2[r, r, :] = 1.0
    bcm2 = bcm2.reshape(2, 2 * D).astype(BF16)
    bcmg = np.zeros((4, FPC, D), dtype=np.float32)
    for j in range(FPC):
        bcmg[j, j, :] = g1
    bcmg = bcmg.reshape(4, FPC * D).astype(BF16)

    in_maps = []
    for i in range(NCORES):
        js = slice(i * FPC, (i + 1) * FPC)
        catT = np.ascontiguousarray(
            cat[:, js, :].transpose(1, 2, 0)                  # [FPC, D, B]
        ).reshape(FPC * D, B).astype(BF16)
        embT = np.ascontiguousarray(
            emb[js].transpose(0, 2, 1)                        # [FPC, D, C]
        ).reshape(FPC * D, C).astype(BF16)
        wqT = np.ascontiguousarray(
            wq[js].transpose(0, 2, 1)                         # [FPC, E, D] (Wq_j^T)
        ).reshape(FPC * D, D).astype(BF16)
        m = {
            "catT": catT,
            "embT": embT,
            "wqT": wqT,
            "wk": wk[js].reshape(FPC * D, D).astype(BF16),
            "wv": wv[js].reshape(FPC * D, D).astype(BF16),
            "w1": w1[js].reshape(FPC * D, H).astype(BF16),
            "w2": w2[js].reshape(FPC * H, D).astype(BF16),
            "wsT": np.ascontiguousarray(ws[js].T).astype(BF16),   # [D, FPC]
            "g1b1": g1b1,
            "g2": g2c,
            "beta2": be2c,
            "b1": np.ascontiguousarray(b1[js]),
            "b2": np.ascontiguousarray(b2[js]),
            "bs": np.ascontiguousarray(bs[js])[:, None],
            "bcm": bcm,
            "bcm2": bcm2,
            "bcmg": bcmg,
        }
        in_maps.append(m)
    return in_maps


def _install_ntff_shim():
    """Provide antenv.axon_hooks (missing in this image) so trace=True can
    capture NTFF profiles via the libaxon ctypes hook."""
    import types

    try:
        from antenv import axon_hooks  # noqa: F401
        return
    except ImportError:
        pass
    import antenv

    mod = types.ModuleType("antenv.axon_hooks")
    _hook = [None]
    mod.set_axon_ntff_profile_hook = lambda h: _hook.__setitem__(0, h)
    mod.get_axon_ntff_profile_hook = lambda: _hook[0]
    sys.modules["antenv.axon_hooks"] = mod
    antenv.axon_hooks = mod
    try:
        sys.path.insert(0, "/root/.axon_site")
        from trn_agent_boot.trn_boot import _ntff_profile_via_ctypes

        mod.set_axon_ntff_profile_hook(
            _ntff_profile_via_ctypes("/opt/axon/libaxon_pjrt.so")
        )
    except Exception as e:  # degrade to no-trace
        print(f"ntff shim: hook unavailable ({e})", file=sys.stderr)


def kernel(**inputs):
    from concourse import bass_utils

    _install_ntff_shim()
    zb = (
        not np.any(np.asarray(inputs["b1"]))
        and not np.any(np.asarray(inputs["b2"]))
        and not np.any(np.asarray(inputs["ln1_b"]))
    )
    nc = _get_program(zb)
    in_maps = _shard_inputs(inputs)
    trace = bool(int(os.environ.get("KERNEL_TRACE", "0")))
    res = bass_utils.run_bass_kernel_spmd(
        nc, in_maps, core_ids=list(range(NCORES)), trace=trace
    )
    LAST["exec_time_ns"] = res.exec_time_ns
    LAST["profile_json"] = res.profile_json
    out = np.empty((B, NC), dtype=np.float32)
    for i in range(NCORES):
        out[:, i * FPC : (i + 1) * FPC] = res.results[i]["out"].T
    return out


# revision 25
# speedup vs baseline: 1.2564x; 1.2564x over previous
"""Trainium2 Bass kernel for nn_C2D_34419867910289.

Computation (per feature j of 32, batch B=4096):
  q = cat_j @ Wq_j ; k = emb_j @ Wk_j ; v = emb_j @ Wv_j
  alpha = softmax(q k^T / sqrt(D)) ; h = LN1(cat_j + alpha v)
  h2 = LN2(h + relu(h W1 + b1) W2 + b2) ; out = sigmoid(h2 . Ws_j + bs_j)

Sharding: Nc (feature) axis across 8 cores, 4 features/core, full batch.
Feature-major dataflow: activations live as [D=128 partitions, Bt=512 free].

v2 design (vs v1 baseline):
 - Single activation-table set for the whole main loop: LN rstd computed as
   exp(-0.5*ln(var+eps)), so Exp/Ln/Relu/Copy all come from
   natural_log_exp_and_others (one ACT_TABLE_LOAD, plus one for the final
   Sigmoid).
 - Scores for both c-chunks land in one 2-bank PSUM slot -> single Exp
   ACTIVATE (FD=1024) per feature; same for ff1+Relu.
 - PSUM slot reuse: hu accumulates into the scores slot half after Exp
   drains it; ff2 accumulates into the ff1 slot half after Relu.
 - All per-tile stats share ONE PSUM bank in 32-row strips with temporal
   reuse: mu1@0-3, sq1@32-35, se(pair0)@64-67, then mu2@0-3/wsy@4-7,
   sq2@32-35.  se(pair1) borrows a broadcast-pool slot's rows 64-67.
 - scalar_tensor_tensor fusions: x1ln = (zc*g1)*rsb, x2 = (x1ln+bb)+ff2.
 - LN2 stats staged to SBUF via DMA directly from PSUM (no ScalarE copies).
 - Deferred LN2 + sigmoid batched chain at kernel end (as v1).
"""

import os
import sys

import numpy as np

sys.path.insert(0, "/opt/trn_rl_repo")

import ml_dtypes

BF16 = ml_dtypes.bfloat16

B, NC, D, C, H = 4096, 32, 128, 256, 256
NCORES = 8
FPC = NC // NCORES  # features per core = 4
BT = 512            # batch tile (matmul moving free dim)
NT = B // BT        # 8 b-tiles
EPS = 1e-5
ISCALE = 1.0 / np.sqrt(np.float32(D))

_CACHE = {}
LAST = {}  # exec_time_ns etc. for test harness


def _build_program(zb):
    """Emit the SPMD per-core Bass/Tile program (identical on all cores)."""
    import concourse.bacc as bacc
    import concourse.bass as bass
    import concourse.tile as tile
    from concourse import bass_isa, mybir

    f32 = mybir.dt.float32
    bf16 = mybir.dt.bfloat16
    AF = mybir.ActivationFunctionType
    OP = mybir.AluOpType

    nc = bacc.Bacc("TRN2", target_bir_lowering=False, debug=False)

    # ---- DRAM I/O (per-core shards) ----
    catT_d = nc.dram_tensor("catT", [FPC * D, B], bf16, kind="ExternalInput")
    embT_d = nc.dram_tensor("embT", [FPC * D, C], bf16, kind="ExternalInput")
    wqT_d = nc.dram_tensor("wqT", [FPC * D, D], bf16, kind="ExternalInput")
    wk_d = nc.dram_tensor("wk", [FPC * D, D], bf16, kind="ExternalInput")
    wv_d = nc.dram_tensor("wv", [FPC * D, D], bf16, kind="ExternalInput")
    w1_d = nc.dram_tensor("w1", [FPC * D, H], bf16, kind="ExternalInput")
    w2_d = nc.dram_tensor("w2", [FPC * H, D], bf16, kind="ExternalInput")
    wsT_d = nc.dram_tensor("wsT", [D, FPC], bf16, kind="ExternalInput")
    g1b1_d = nc.dram_tensor("g1b1", [D, 2], f32, kind="ExternalInput")
    g2_d = nc.dram_tensor("g2", [D, 1], f32, kind="ExternalInput")
    beta2_d = nc.dram_tensor("beta2", [D, 1], bf16, kind="ExternalInput")
    b1_d = nc.dram_tensor("b1", [FPC, H], f32, kind="ExternalInput")
    b2_d = nc.dram_tensor("b2", [FPC, D], f32, kind="ExternalInput")
    bs_d = nc.dram_tensor("bs", [FPC, 1], f32, kind="ExternalInput")
    bcm_d = nc.dram_tensor("bcm", [4, FPC * D], bf16, kind="ExternalInput")
    bcm2_d = nc.dram_tensor("bcm2", [2, 2 * D], bf16, kind="ExternalInput")
    bcmg_d = nc.dram_tensor("bcmg", [4, FPC * D], bf16, kind="ExternalInput")
    out_d = nc.dram_tensor("out", [FPC, B], f32, kind="ExternalOutput")

    with tile.TileContext(nc) as tc:
        with (
            tc.tile_pool(name="const", bufs=1) as constp,
            tc.tile_pool(name="wtmp", bufs=2) as wtmp,
            tc.tile_pool(name="cat", bufs=12) as catp,
            tc.tile_pool(name="et", bufs=4) as etp,
            tc.tile_pool(name="x1p", bufs=6) as x1p,
            tc.tile_pool(name="xln", bufs=3) as xlnp,
            tc.tile_pool(name="rp", bufs=2) as rp,
            tc.tile_pool(name="x2p", bufs=3) as x2p,
            tc.tile_pool(name="tmp", bufs=4) as tmpp,
            tc.tile_pool(name="stash", bufs=4) as stashp,
            tc.tile_pool(name="finp", bufs=1) as finp,
            tc.tile_pool(name="sp", bufs=2, space="PSUM") as spp,
            tc.tile_pool(name="bc", bufs=2, space="PSUM") as bcp,
            tc.tile_pool(name="pst", bufs=2, space="PSUM") as pstp,
        ):
            # ---------------- constants ----------------
            ones_c1 = constp.tile([D, 1], bf16, tag="c_ones")
            nc.vector.memset(ones_c1, 1.0)
            warm_rhs = constp.tile([D, BT], bf16, tag="c_warm")
            nc.vector.memset(warm_rhs, 0.0)
            warm_ps = bcp.tile([4, BT], f32, tag="bc")
            for _ in range(16):
                nc.tensor.matmul(
                    warm_ps, ones_c1[:, 0:1].broadcast_to([D, 4]), warm_rhs,
                    start=True, stop=True, skip_group_check=True,
                )
            epsT = constp.tile([D, 1], f32, tag="c_eps")
            nc.vector.memset(epsT, EPS)

            # masked stat lhsTs (per feature j):
            # m4[j]:  [D,4] col j = 1/128   -> mu rows
            # se4[j]: [D,4] col j = 1.0     -> sumexp rows
            # mw8[j]: [D,8] col j = 1/128, col 4+j = Ws*g2 -> mu2 rows 0..3,
            #         wsy rows 4..7 (one matmul on x2)
            m4, se4, mw8 = [], [], []
            for j in range(FPC):
                t = constp.tile([D, 4], bf16, tag=f"c_m4_{j}")
                nc.vector.memset(t, 0.0)
                nc.vector.memset(t[:, j : j + 1], 1.0 / 128.0)
                m4.append(t)
                t = constp.tile([D, 4], bf16, tag=f"c_se4_{j}")
                nc.vector.memset(t, 0.0)
                nc.vector.memset(t[:, j : j + 1], 1.0)
                se4.append(t)
                t = constp.tile([D, 8], bf16, tag=f"c_mw8_{j}")
                nc.vector.memset(t, 0.0)
                nc.vector.memset(t[:, j : j + 1], 1.0 / 128.0)
                mw8.append(t)
            # M=36 zero-init variant of the mu mask: first matmul of each LN1
            # bank clears rows 0..35 so accumulates are defined
            m36_0 = constp.tile([D, 36], bf16, tag="c_m36")
            nc.vector.memset(m36_0, 0.0)
            nc.vector.memset(m36_0[:, 0:1], 1.0 / 128.0)

            # one-hot row-broadcast lhsT (row j ones in [4, D] slice j)
            bcm = constp.tile([4, FPC * D], bf16, tag="c_bcm")
            nc.sync.dma_start(bcm, bcm_d[:, :])

            def bc(j):
                return bcm[:, j * D : (j + 1) * D]

            bcm2 = constp.tile([2, 2 * D], bf16, tag="c_bcm2")
            nc.sync.dma_start(bcm2, bcm2_d[:, :])
            bcmg = constp.tile([4, FPC * D], bf16, tag="c_bcmg")
            nc.sync.dma_start(bcmg, bcmg_d[:, :])

            def bcg(j):
                return bcmg[:, j * D : (j + 1) * D]

            def bc2(r):
                return bcm2[:, r * D : (r + 1) * D]

            def rowbc(row_ap):
                """[1,BT] SBUF row -> [1,D,BT] AP replicating the row D times
                via a 0-stride middle dim (DMA row-broadcast source)."""
                return bass.AP(
                    tensor=row_ap.tensor, offset=row_ap.offset,
                    ap=[list(row_ap.ap[0]), [0, D], [1, BT]],
                )

            # small param cols
            g1b1 = constp.tile([D, 2], f32, tag="c_g1b1")
            nc.sync.dma_start(g1b1, g1b1_d[:, :])
            negg1d = constp.tile([D, 1], f32, tag="c_negg1d")
            nc.vector.tensor_scalar(
                negg1d, g1b1[:, 0:1], -1.0 / 128.0, None, OP.mult
            )
            g2c = constp.tile([D, 1], f32, tag="c_g2")
            nc.sync.dma_start(g2c, g2_d[:, :])
            beta2c = constp.tile([D, 1], bf16, tag="c_beta2")
            nc.sync.dma_start(beta2c, beta2_d[:, :])
            wsT = constp.tile([D, FPC], bf16, tag="c_wsT")
            nc.sync.dma_start(wsT, wsT_d[:, :])
            bs_sb = constp.tile([FPC, 1], f32, tag="c_bs")
            nc.sync.dma_start(bs_sb, bs_d[:, :])
            Scol = constp.tile([4, 1], f32, tag="c_Scol")
            Tcol = constp.tile([4, 1], f32, tag="c_Tcol")
            NH = 2 * NT  # rows per half (4 features x NT/2 tiles)
            Scol32 = [
                constp.tile([NH, 1], f32, tag=f"c_Scol32_{h}", name=f"Scol32_{h}") for h in range(2)
            ]
            Tcol32 = [
                constp.tile([NH, 1], f32, tag=f"c_Tcol32_{h}", name=f"Tcol32_{h}") for h in range(2)
            ]

            # packed deferred-LN2 stats, split in half-kernel chunks;
            # row index = 4*(t % (NT/2)) + j within each half
            NR = 4 * NT
            fin_mu2 = [finp.tile([NH, BT], f32, tag=f"fin_mu2_{h}", name=f"fin_mu2_{h}") for h in range(2)]
            fin_wsy = [finp.tile([NH, BT], f32, tag=f"fin_wsy_{h}", name=f"fin_wsy_{h}") for h in range(2)]
            fin_q = [finp.tile([NH, BT], f32, tag=f"fin_q_{h}", name=f"fin_q_{h}") for h in range(2)]

            # ---------------- per-feature setup ----------------
            mq_s, v_s, w1_s, w2_s, c1_s, bb_s = [], [], [], [], [], []
            for j in range(FPC):
                r0 = j * D
                w1 = constp.tile([D, H], bf16, tag=f"w1{j}")
                nc.scalar.dma_start(w1, w1_d[r0 : r0 + D, :])
                w1_s.append(w1)
                w2 = constp.tile([D, 2, D], bf16, tag=f"w2{j}")
                nc.scalar.dma_start(w2[:, 0, :], w2_d[j * H : j * H + D, :])
                nc.scalar.dma_start(w2[:, 1, :], w2_d[j * H + D : j * H + 2 * D, :])
                w2_s.append(w2)
                b1c = wtmp.tile([D, 2], f32, tag="b1t")
                nc.sync.dma_start(
                    b1c, bass.AP(tensor=b1_d, offset=j * H, ap=[[1, D], [D, 2]])
                )
                b2c = wtmp.tile([D, 1], f32, tag="b2t")
                nc.sync.dma_start(
                    b2c, bass.AP(tensor=b2_d, offset=j * D, ap=[[1, D], [D, 1]])
                )
                bb = constp.tile([D, 1], f32, tag=f"bb{j}")
                nc.vector.tensor_add(bb, g1b1[:, 1:2], b2c)  # beta1 + b2
                bb_s.append(bb)

                embT = wtmp.tile([D, C], bf16, tag="embT")
                nc.gpsimd.dma_start(embT, embT_d[r0 : r0 + D, :])
                wk = wtmp.tile([D, D], bf16, tag="wk")
                nc.gpsimd.dma_start(wk, wk_d[r0 : r0 + D, :])
                wv = wtmp.tile([D, D], bf16, tag="wv")
                nc.scalar.dma_start(wv, wv_d[r0 : r0 + D, :])
                wqT = wtmp.tile([D, D], bf16, tag="wqT")
                nc.gpsimd.dma_start(wqT, wqT_d[r0 : r0 + D, :])

                # beta1 column in bf16 for the c1 matvec
                be1_16 = wtmp.tile([D, 1], bf16, tag="be1_16")
                nc.vector.tensor_copy(be1_16, g1b1[:, 1:2])

                # kT = Wk.T @ embT -> [E, C], scaled by 1/sqrt(D)
                kps = spp.tile([D, 2, BT], f32, tag="sp")
                nc.tensor.matmul(kps[:, 0, :C], wk, embT, start=True, stop=True)
                kts = wtmp.tile([D, C], bf16, tag="kts")
                nc.scalar.activation(kts, kps[:, 0, :C], AF.Copy, scale=float(ISCALE))

                # M_j = Wq_j @ kts -> [D, C]; scores^T = M_j.T @ cat^T
                mps = spp.tile([D, 2, BT], f32, tag="sp")
                nc.tensor.matmul(mps[:, 0, :C], wqT, kts, start=True, stop=True)
                mq = constp.tile([D, C], bf16, tag=f"mq{j}")
                nc.scalar.activation(mq, mps[:, 0, :C], AF.Copy)
                mq_s.append(mq)

                # v chunks: [c-chunk=128, E]
                vt = constp.tile([D, 2, D], bf16, tag=f"v{j}")
                vps = spp.tile([D, 2, BT], f32, tag="sp")
                for c in range(2):
                    nc.tensor.matmul(
                        vps[:, c, :D], embT[:, c * D : (c + 1) * D], wv,
                        start=True, stop=True,
                    )
                nc.scalar.activation(vt, vps[:, :, :D], AF.Copy)
                v_s.append(vt)

                # c1[hc] = b1[hc] + W1_hc^T beta1  (relu bias, per h-chunk)
                c1ps = bcp.tile([D, BT], f32, tag="bc")
                for hc in range(2):
                    nc.tensor.matmul(
                        c1ps[:, hc : hc + 1],
                        w1[:, hc * D : (hc + 1) * D], be1_16,
                        start=True, stop=True,
                    )
                c1c = constp.tile([D, 2], f32, tag=f"c1{j}")
                nc.vector.tensor_add(c1c, b1c, c1ps[:, 0:2])
                c1_s.append(c1c)

            # Wsg2 = Ws*g2 ; S_j = sum_d Wsg2_j ; T_j = Ws_j.beta2 + bs_j
            wsg2_4 = constp.tile([D, FPC], bf16, tag="c_wsg2")
            nc.vector.tensor_scalar(wsg2_4, wsT, g2c, None, OP.mult)
            for j in range(FPC):
                nc.gpsimd.tensor_copy(mw8[j][:, 4 + j : 5 + j], wsg2_4[:, j : j + 1])
            sps = bcp.tile([FPC, BT], f32, tag="bc")
            nc.tensor.matmul(sps[:, :1], wsg2_4, ones_c1, start=True, stop=True)
            nc.scalar.activation(Scol, sps[:, :1], AF.Copy)
            tps = bcp.tile([FPC, BT], f32, tag="bc")
            nc.tensor.matmul(tps[:, :1], wsT, beta2c, start=True, stop=True)
            tcol0 = constp.tile([FPC, 1], f32, tag="c_T0")
            nc.scalar.activation(tcol0, tps[:, :1], AF.Copy)
            nc.gpsimd.tensor_add(Tcol, tcol0, bs_sb)
            for t in range(NT):
                h, r = t // (NT // 2), 4 * (t % (NT // 2))
                nc.sync.dma_start(Scol32[h][r : r + 4, :], Scol)
                nc.sync.dma_start(Tcol32[h][r : r + 4, :], Tcol)

            # deferred LN2 + sigmoid for half h of the b-tiles
            def fin_chain(h):
                n = NH
                musq2 = stashp.tile([n, BT], f32, tag="musq2")
                nc.vector.tensor_mul(musq2, fin_mu2[h], fin_mu2[h])
                var2 = stashp.tile([n, BT], f32, tag="var2")
                nc.vector.tensor_sub(var2, fin_q[h], musq2)
                std2 = stashp.tile([n, BT], f32, tag="std2")
                nc.scalar.activation(std2, var2, AF.Sqrt, bias=epsT[0:n, :])
                rstd2 = stashp.tile([n, BT], f32, tag="rstd2")
                nc.vector.reciprocal_approx_fast(rstd2, std2)
                mu2S = stashp.tile([n, BT], f32, tag="mu2S")
                nc.vector.tensor_scalar(
                    mu2S, fin_mu2[h], Scol32[h], None, OP.mult
                )
                t1 = stashp.tile([n, BT], f32, tag="t1")
                nc.vector.tensor_sub(t1, fin_wsy[h], mu2S)
                t2 = stashp.tile([n, BT], f32, tag="t2")
                nc.vector.tensor_mul(t2, t1, rstd2)
                o32 = stashp.tile([n, BT], f32, tag="o32")
                nc.scalar.activation(o32, t2, AF.Sigmoid, bias=Tcol32[h])
                # row 4t'+j -> out[j, 512t : 512t+512]
                out_ap = bass.AP(
                    tensor=out_d, offset=h * (NT // 2) * BT,
                    ap=[[BT, NT // 2], [B, FPC], [1, BT]],
                )
                nc.sync.dma_start(out_ap, o32)

            # ---------------- main loop over b-tiles ----------------
            for t in range(NT):
                b0 = t * BT
                # stats bank strips (temporal reuse):
                #   rows 0-3 mu1 (m36 clears 0-35), rows 32-35 sq1,
                #   rows 64-67 se(pair0); later mu2@0-3/wsy@4-7, sq2@32-35
                stats = pstp.tile([D, BT], f32, tag="st")

                cat_sb = [None] * FPC
                mu_t = [None] * FPC
                et_sb = [None] * FPC
                sp_sl = [None] * FPC
                x1_sb = [None] * FPC
                se_ps = [None] * 2      # psum holding se rows for pair p
                se_r0 = [0, 0]          # row offset of se rows in that psum
                srows = [None] * 2      # [4,BT] SBUF with s at rows 2p..2p+1

                # ---- phase A: scores, exp, se, hu (per feature) ----
                for j in range(FPC):
                    p = j // 2
                    ct = catp.tile([D, BT], bf16, tag="cat")
                    nc.gpsimd.dma_start(
                        ct, catT_d[j * D : (j + 1) * D, b0 : b0 + BT]
                    )
                    cat_sb[j] = ct
                    slot = spp.tile([D, 2, BT], f32, tag="sp")
                    sp_sl[j] = slot
                    if j == 0:
                        se_ps[0] = stats
                        se_r0[0] = 64
                    elif j == 2:
                        # pair1 sumexp -> stats rows 96-99 (strip 3, unused)
                        se_ps[1] = stats
                        se_r0[1] = 96
                    for c in range(2):
                        nc.tensor.matmul(
                            slot[:, c, :], mq_s[j][:, c * D : (c + 1) * D], ct,
                            start=True, stop=True,
                        )
                    et = etp.tile([D, 2, BT], bf16, tag="et")
                    nc.scalar.activation(et, slot, AF.Exp)
                    et_sb[j] = et
                    r0 = se_r0[p]
                    for c in range(2):
                        nc.tensor.matmul(
                            se_ps[p][r0 : r0 + 4, :], se4[j % 2], et[:, c, :],
                            start=(j % 2 == 0 and c == 0),
                            stop=(j % 2 == 1 and c == 1),
                            tile_position=(0, r0),
                            skip_group_check=True,
                        )
                    # hu reuses the scores slot half c=0 (start clears it)
                    for c in range(2):
                        nc.tensor.matmul(
                            slot[:, 0, :], v_s[j][:, c, :], et[:, c, :],
                            start=(c == 0), stop=(c == 1),
                        )
                    if j % 2 == 1:
                        # s for this pair -> srows[p] [2,BT] (bf16)
                        sr = stashp.tile([2, BT], bf16, tag=f"sr{p}")
                        nc.vector.tensor_copy(sr, se_ps[p][r0 : r0 + 2, :])
                        srows[p] = sr

                # ---- phase B: x1 = cat*s + hu, LN1 stats (per feature) ----
                for j in range(FPC):
                    p = j // 2
                    sbb = bcp.tile([D, BT], f32, tag="bc")
                    nc.tensor.matmul(
                        sbb, bc2(j % 2), srows[p], start=True, stop=True
                    )
                    cs = tmpp.tile([D, BT], bf16, tag="cs")
                    nc.vector.tensor_mul(cs, cat_sb[j], sbb)
                    x1 = x1p.tile([D, BT], bf16, tag="x1")
                    nc.vector.tensor_add(x1, cs, sp_sl[j][:, 0, :])
                    x1_sb[j] = x1
                    if zb:
                        # LN1 variance eliminated (LN2 absorbs column scale)
                        nc.tensor.matmul(
                            stats[0:4, :], m4[j], x1,
                            start=(j == 0), stop=(j == FPC - 1),
                            skip_group_check=True,
                        )
                        continue
                    sq1 = tmpp.tile([D, BT], bf16, tag="sq1")
                    nc.gpsimd.tensor_mul(sq1, x1, x1)
                    if j == 0:
                        nc.tensor.matmul(
                            stats[0:36, :], m36_0, x1,
                            start=True, stop=False,
                            skip_group_check=True,
                        )
                    else:
                        nc.tensor.matmul(
                            stats[0:4, :], m4[j], x1,
                            start=False, stop=False,
                            skip_group_check=True,
                        )
                    nc.tensor.matmul(
                        stats[32:36, :], m4[j], sq1,
                        start=False, stop=(j == FPC - 1),
                        tile_position=(0, 32),
                        skip_group_check=True,
                    )

                # ---- LN1 batched stat chain ----
                muS = stashp.tile([4, BT], bf16, tag="muS")
                nc.scalar.activation(muS, stats[0:4, :], AF.Copy)
                if zb:
                    rstd = None
                else:
                    musq = stashp.tile([4, BT], bf16, tag="musq")
                    nc.vector.tensor_mul(musq, muS, muS)
                    var1 = stashp.tile([4, BT], f32, tag="var1")
                    nc.vector.tensor_sub(var1, stats[32:36, :], musq)
                    rstd = stashp.tile([4, BT], bf16, tag="rstd")
                    std1 = stashp.tile([4, BT], f32, tag="std1")
                    nc.scalar.activation(std1, var1, AF.Sqrt, bias=epsT[0:4, :])
                    rstdf = stashp.tile([4, BT], f32, tag="rstdf")
                    nc.vector.reciprocal_approx_fast(rstdf, std1)
                    nc.vector.tensor_copy(rstd, rstdf)

                # ---- phase C: LN1 apply, FFN, LN2 stats ----
                for j in range(FPC):
                    if zb:
                        # u = x1*g1 - mu*g1 = LN1(x1)*std1; ReLU is positively
                        # homogeneous and LN2 is column-scale invariant, so
                        # the std1 factor never needs to be applied.
                        mub = bcp.tile([D, BT], f32, tag="bc")
                        nc.tensor.matmul(mub, bcg(j), muS, start=True, stop=True)
                        u = xlnp.tile([D, BT], bf16, tag="x1ln")
                        nc.vector.scalar_tensor_tensor(
                            u, x1_sb[j], g1b1[:, 0:1], mub, OP.mult, OP.subtract
                        )
                        x1ln = u
                    else:
                        mub = bcp.tile([D, BT], f32, tag="bc")
                        nc.tensor.matmul(mub, bc(j), muS, start=True, stop=True)
                        rsb = bcp.tile([D, BT], f32, tag="bc")
                        nc.tensor.matmul(rsb, bc(j), rstd, start=True, stop=True)
                        zc = tmpp.tile([D, BT], bf16, tag="zc")
                        nc.vector.tensor_sub(zc, x1_sb[j], mub)
                        x1ln = xlnp.tile([D, BT], bf16, tag="x1ln")
                        nc.vector.scalar_tensor_tensor(
                            x1ln, zc, g1b1[:, 0:1], rsb, OP.mult, OP.mult
                        )
                    fslot = spp.tile([D, 2, BT], f32, tag="sp")
                    for hc in range(2):
                        nc.tensor.matmul(
                            fslot[:, hc, :], w1_s[j][:, hc * D : (hc + 1) * D],
                            x1ln, start=True, stop=True,
                        )
                    r_sb = rp.tile([D, 2, BT], bf16, tag="r")
                    if zb:
                        nc.scalar.activation(r_sb, fslot, AF.Relu)
                    else:
                        for hc in range(2):
                            nc.scalar.activation(
                                r_sb[:, hc, :], fslot[:, hc, :], AF.Relu,
                                bias=c1_s[j][:, hc : hc + 1],
                            )
                    # ff2 accumulates into fslot half c=0 (start clears)
                    for hc in range(2):
                        nc.tensor.matmul(
                            fslot[:, 0, :], w2_s[j][:, hc, :], r_sb[:, hc, :],
                            start=(hc == 0), stop=(hc == 1),
                        )
                    x2 = x2p.tile([D, BT], bf16, tag="x2")
                    if zb:
                        # X = u + ff2 = x2 * std1 (LN2 absorbs the scale)
                        nc.vector.tensor_add(x2, x1ln, fslot[:, 0, :])
                    else:
                        nc.vector.scalar_tensor_tensor(
                            x2, x1ln, bb_s[j], fslot[:, 0, :], OP.add, OP.add
                        )
                    sq2 = tmpp.tile([D, BT], bf16, tag="sq2")
                    nc.gpsimd.tensor_mul(sq2, x2, x2)
                    # mu2 rows 0-3 / wsy rows 4-7 (strip 0 reuse after muS read)
                    nc.tensor.matmul(
                        stats[0:8, :], mw8[j], x2,
                        start=(j == 0), stop=(j == FPC - 1),
                        skip_group_check=True,
                    )
                    # sq2 rows 32-35 (strip 32 reuse after var1 read)
                    nc.tensor.matmul(
                        stats[32:36, :], m4[j], sq2,
                        start=(j == 0), stop=(j == FPC - 1),
                        tile_position=(0, 32),
                        skip_group_check=True,
                    )

                # stage LN2 stats to SBUF, then DMA into packed fin buffers
                stage = stashp.tile([8, BT], f32, tag="stage")
                nc.scalar.activation(stage, stats[0:8, :], AF.Copy)
                stage2 = stashp.tile([4, BT], f32, tag="stage2")
                nc.scalar.activation(stage2, stats[32:36, :], AF.Copy)
                h, r = t // (NT // 2), 4 * (t % (NT // 2))
                nc.sync.dma_start(fin_mu2[h][r : r + 4, :], stage[0:4, :])
                nc.sync.dma_start(fin_wsy[h][r : r + 4, :], stage[4:8, :])
                nc.sync.dma_start(fin_q[h][r : r + 4, :], stage2)
                if t == NT - 1:
                    fin_chain(0)
                    fin_chain(1)



    nc.compile()
    return nc


def _get_program(zb=True):
    key = f"nc{int(bool(zb))}"
    if key not in _CACHE:
        _CACHE[key] = _build_program(zb)
    return _CACHE[key]


def _shard_inputs(inputs):
    """Host-side layout prep: shard by feature, transpose, cast. No FLOPs."""
    cat = np.ascontiguousarray(np.asarray(inputs["cat_vecs"], dtype=np.float32))
    emb = np.asarray(inputs["embed_weights"], dtype=np.float32)
    wq = np.asarray(inputs["Wq"], dtype=np.float32)
    wk = np.asarray(inputs["Wk"], dtype=np.float32)
    wv = np.asarray(inputs["Wv"], dtype=np.float32)
    w1 = np.asarray(inputs["W1"], dtype=np.float32)
    w2 = np.asarray(inputs["W2"], dtype=np.float32)
    b1 = np.asarray(inputs["b1"], dtype=np.float32)
    b2 = np.asarray(inputs["b2"], dtype=np.float32)
    ws = np.asarray(inputs["Ws"], dtype=np.float32)
    bs = np.asarray(inputs["bs"], dtype=np.float32)
    g1 = np.asarray(inputs["ln1_g"], dtype=np.float32)
    be1 = np.asarray(inputs["ln1_b"], dtype=np.float32)
    g2 = np.asarray(inputs["ln2_g"], dtype=np.float32)
    be2 = np.asarray(inputs["ln2_b"], dtype=np.float32)

    g1b1 = np.ascontiguousarray(np.stack([g1, be1], axis=1))  # [D,2] f32
    g2c = np.ascontiguousarray(g2[:, None])
    be2c = be2[:, None].astype(BF16)
    bcm = np.zeros((4, FPC, D), dtype=np.float32)
    for j in range(FPC):
        bcm[j, j, :] = 1.0
    bcm = bcm.reshape(4, FPC * D).astype(BF16)
    bcm2 = np.zeros((2, 2, D), dtype=np.float32)
    for r in range(2):
        bcm2[r, r, :] = 1.0
    bcm2 = bcm2.reshape(2, 2 * D).astype(BF16)
    bcmg = np.zeros((4, FPC, D), dtype=np.float32)
    for j in range(FPC):
        bcmg[j, j, :] = g1
    bcmg = bcmg.reshape(4, FPC * D).astype(BF16)

    in_maps = []
    for i in range(NCORES):
        js = slice(i * FPC, (i + 1) * FPC)
        catT = np.ascontiguousarray(
            cat[:, js, :].transpose(1, 2, 0)                  # [FPC, D, B]
        ).reshape(FPC * D, B).astype(BF16)
        embT = np.ascontiguousarray(
            emb[js].transpose(0, 2, 1)                        # [FPC, D, C]
        ).reshape(FPC * D, C).astype(BF16)
        wqT = np.ascontiguousarray(
            wq[js].transpose(0, 2, 1)                         # [FPC, E, D] (Wq_j^T)
        ).reshape(FPC * D, D).astype(BF16)
        m = {
            "catT": catT,
            "embT": embT,
            "wqT": wqT,
            "wk": wk[js].reshape(FPC * D, D).astype(BF16),
            "wv": wv[js].reshape(FPC * D, D).astype(BF16),
            "w1": w1[js].reshape(FPC * D, H).astype(BF16),
            "w2": w2[js].reshape(FPC * H, D).astype(BF16),
            "wsT": np.ascontiguousarray(ws[js].T).astype(BF16),   # [D, FPC]
            "g1b1": g1b1,
            "g2": g2c,
            "beta2": be2c,
            "b1": np.ascontiguousarray(b1[js]),
            "b2": np.ascontiguousarray(b2[js]),
            "bs": np.ascontiguousarray(bs[js])[:, None],
            "bcm": bcm,
            "bcm2": bcm2,
            "bcmg": bcmg,
        }
        in_maps.append(m)
    return in_maps


def _install_ntff_shim():
    """Provide antenv.axon_hooks (missing in this image) so trace=True can
    capture NTFF profiles via the libaxon ctypes hook."""
    import types

    try:
        from antenv import axon_hooks  # noqa: F401
        return
    except ImportError:
        pass
    import antenv

    mod = types.ModuleType("antenv.axon_hooks")
    _hook = [None]
    mod.set_axon_ntff_profile_hook = lambda h: _hook.__setitem__(0, h)
    mod.get_axon_ntff_profile_hook = lambda: _hook[0]
    sys.modules["antenv.axon_hooks"] = mod
    antenv.axon_hooks = mod
    try:
        sys.path.insert(0, "/root/.axon_site")
        from trn_agent_boot.trn_boot import _ntff_profile_via_ctypes

        mod.set_axon_ntff_profile_hook(
            _ntff_profile_via_ctypes("/opt/axon/libaxon_pjrt.so")
        )
    except Exception as e:  # degrade to no-trace
        print(f"ntff shim: hook unavailable ({e})", file=sys.stderr)


def kernel(**inputs):
    from concourse import bass_utils

    _install_ntff_shim()
    zb = (
        not np.any(np.asarray(inputs["b1"]))
        and not np.any(np.asarray(inputs["b2"]))
        and not np.any(np.asarray(inputs["ln1_b"]))
    )
    nc = _get_program(zb)
    in_maps = _shard_inputs(inputs)
    trace = bool(int(os.environ.get("KERNEL_TRACE", "0")))
    res = bass_utils.run_bass_kernel_spmd(
        nc, in_maps, core_ids=list(range(NCORES)), trace=trace
    )
    LAST["exec_time_ns"] = res.exec_time_ns
    LAST["profile_json"] = res.profile_json
    out = np.empty((B, NC), dtype=np.float32)
    for i in range(NCORES):
        out[:, i * FPC : (i + 1) * FPC] = res.results[i]["out"].T
    return out
